# revision 12
# baseline (speedup 1.0000x reference)
"""Trainium2 Bass kernel for nn_DetectorKe_652835029279 (Gaussian-mixture
log-likelihood detector: weighted logsumexp over 256 Mahalanobis distances).

v2 "squares basis": ll_i = logsumexp_j(-0.5 x^T A_j x + x.(A_j c_j) + bias_j)
with the quadratic form expanded in the basis
  (x_a + x_b)^2  for pairs at circular distance k=1..15  (4 chunks of 128)
  (2 x_d)^2      for the diagonal (k=0 slots)
  x_d * x_{d+16} for the 16 distance-16 pairs            (chunk 4, rows 0:16)
  x_d, 1         linear + bias rows                      (chunk 4, rows 32:65)
so the selection matmuls produce SUMS x_a+x_b directly (2-hot SEL), the
elementwise step is a unary SQUARE (splittable between ACT and DVE), and the
distance-16 pairs come from one cheap DVE bf16 multiply of two SBUF tiles.
The whole pipeline is bf16 (FWL hides LDWEIGHTS behind the matmul stream);
X arrives pre-transposed [32, N] in bf16 from the host, which removes all
PE transposes. d' = G^T @ U with 5 chunks (4x K=128 + 1x K=65): 20 main
matmuls of N=256 + 4 selection matmuls of N=512 per 512-row tile.

Per tile: 3 DMAs (X^T slab + two 16-row slices) -> DVE k16-product ->
4 SEL matmuls (PSUM) -> squares (3 on ACT, 1 on DVE) -> 20 accumulating
matmuls into one [128,1024] PSUM tile -> ACT exp (bf16) -> DVE free-dim
reduce into per-subtile sums -> (end) Ln + PE transpose + DMA out.
PSUM budget: 4 banks sel-sums + 2x2 banks main = 8.
"""
import sys

if "/opt/trn_rl_repo" not in sys.path:
    sys.path.insert(0, "/opt/trn_rl_repo")

import numpy as np
import ml_dtypes

BF16 = ml_dtypes.bfloat16

N, D, M = 131072, 32, 256
NCORES = 8
NC_ROWS = N // NCORES          # 16384
TILE_ROWS = 512
NTILES = NC_ROWS // TILE_ROWS  # 32
NCHUNK = 5

_PROGRAM = None


def _build_program():
    import concourse.bacc as bacc
    import concourse.mybir as mybir
    import concourse.tile as tile

    f32 = mybir.dt.float32
    bf16 = mybir.dt.bfloat16
    AF = mybir.ActivationFunctionType

    def _exp_reduce(nc, mybir, expp, psm, sums_sb, t, bf16, AF):
        ex = expp.tile([128, 4 * M], bf16, tag="exp")
        nc.scalar.activation(ex[:], psm[:], AF.Exp)
        with nc.allow_low_precision("bf16 exp sums; 0.4% rel on ll is in budget"):
            nc.vector.tensor_reduce(
                sums_sb[:, 4 * t : 4 * t + 4],
                ex[:].rearrange("p (s m) -> p s m", s=4),
                axis=mybir.AxisListType.X,
                op=mybir.AluOpType.add,
            )

    nc = bacc.Bacc(None, target_bir_lowering=False)
    XT_d = nc.dram_tensor("XT", [D, NC_ROWS], bf16, kind="ExternalInput")
    U_d = nc.dram_tensor("U", [128, NCHUNK, M], bf16, kind="ExternalInput")
    SEL_d = nc.dram_tensor("SEL", [33, 4, 128], bf16, kind="ExternalInput")
    Z16_d = nc.dram_tensor("Z16", [16, TILE_ROWS], bf16, kind="ExternalInput")
    ONE1_d = nc.dram_tensor("ONE1", [1, TILE_ROWS], bf16, kind="ExternalInput")
    EYE_d = nc.dram_tensor("EYE", [128, 128], f32, kind="ExternalInput")
    OUT_d = nc.dram_tensor("out", [NC_ROWS], f32, kind="ExternalOutput")

    with tile.TileContext(nc) as tc:
        with (
            tc.tile_pool(name="const", bufs=1) as constp,
            tc.tile_pool(name="c4", bufs=2) as c4pool,
            tc.tile_pool(name="xlo", bufs=2) as xlop,
            tc.tile_pool(name="xhi", bufs=2) as xhip,
            tc.tile_pool(name="ch", bufs=8) as chp,
            tc.tile_pool(name="expp", bufs=2) as expp,
            tc.tile_pool(name="sums", bufs=1) as sumsp,
            tc.tile_pool(name="fin", bufs=1) as finp,
            tc.tile_pool(name="ps_sums", bufs=4, space="PSUM") as ps_sums,
            tc.tile_pool(name="ps_main", bufs=2, space="PSUM") as ps_main,
        ):
            U_sb = constp.tile([128, NCHUNK, M], bf16)
            nc.sync.dma_start(U_sb[:], U_d[:])
            SEL_sb = constp.tile([33, 4, 128], bf16)
            nc.sync.dma_start(SEL_sb[:], SEL_d[:])
            EYE_sb = constp.tile([128, 128], f32)
            nc.sync.dma_start(EYE_sb[:], EYE_d[:])

            sums_sb = sumsp.tile([128, NTILES * 4], bf16)

            # persistent chunk-4 tiles: rows 0:16 = x_d*x_{d+16} (per tile),
            # rows 16:32 = zeros (once), rows 32:64 = X^T (DMA per tile),
            # row 64 = ones (once). Main matmul reads K=0:65.
            c4_tiles = []
            for i in range(2):
                c4t = c4pool.tile(
                    [128, TILE_ROWS], bf16, tag=f"c4{i}", bufs=1, name=f"c4_{i}"
                )
                nc.sync.dma_start(c4t[16:32, :], Z16_d[:])
                nc.sync.dma_start(c4t[64:65, :], ONE1_d[:])
                c4_tiles.append(c4t)

            # sel-matmul moving operand at base partition 0: [X^T; ones]
            xsel_tiles = []
            for i in range(2):
                xs = c4pool.tile(
                    [33, TILE_ROWS], bf16, tag=f"xs{i}", bufs=1, name=f"xs_{i}"
                )
                nc.sync.dma_start(xs[32:33, :], ONE1_d[:])
                xsel_tiles.append(xs)

            prev_psm = None
            for t in range(NTILES):
                c4t = c4_tiles[t % 2]
                xs = xsel_tiles[t % 2]
                lo = t * TILE_ROWS
                nc.sync.dma_start(c4t[32:64, :], XT_d[:, lo : lo + TILE_ROWS])
                nc.sync.dma_start(xs[0:32, :], XT_d[:, lo : lo + TILE_ROWS])
                xlo = xlop.tile([16, TILE_ROWS], bf16, tag="xlo")
                nc.gpsimd.dma_start(xlo[:], XT_d[0:16, lo : lo + TILE_ROWS])
                xhi = xhip.tile([16, TILE_ROWS], bf16, tag="xhi")
                nc.gpsimd.dma_start(xhi[:], XT_d[16:32, lo : lo + TILE_ROWS])

                # distance-16 pair products (bf16 2x DVE)
                nc.vector.tensor_mul(c4t[0:16, :], xlo[:], xhi[:])

                # selection matmuls -> (x_a + x_b) sums in PSUM, then square
                ch_tiles = []
                for c in range(4):
                    sm = ps_sums.tile([128, TILE_ROWS], f32, tag="sm")
                    nc.tensor.matmul(
                        sm[:], SEL_sb[:, c, :], xs[:], start=True, stop=True
                    )
                    ch = chp.tile([128, TILE_ROWS], bf16, tag="ch")
                    if c == 3:
                        # DVE has one PSUM read port: copy to bf16 SBUF first,
                        # then square in 2x bf16 mode.
                        tmp = chp.tile([128, TILE_ROWS], bf16, tag="sq3tmp", bufs=2)
                        nc.vector.tensor_copy(tmp[:], sm[:])
                        nc.vector.tensor_mul(ch[:], tmp[:], tmp[:])
                    else:
                        nc.scalar.activation(ch[:], sm[:], AF.Square)
                    ch_tiles.append(ch)

                # main accumulating matmuls: 4 row-subtiles x 5 chunks
                psm = ps_main.tile([128, 4 * M], f32, tag="main")
                for sub in range(4):
                    for c in range(4):
                        nc.tensor.matmul(
                            psm[:, sub * M : (sub + 1) * M],
                            ch_tiles[c][:, sub * 128 : (sub + 1) * 128],
                            U_sb[:, c, :],
                            start=(c == 0),
                            stop=False,
                        )
                    nc.tensor.matmul(
                        psm[:, sub * M : (sub + 1) * M],
                        c4t[0:65, sub * 128 : (sub + 1) * 128],
                        U_sb[0:65, 4, :],
                        start=False,
                        stop=True,
                    )

                # software-pipelined epilogue: exp/reduce for tile t-1, so the
                # ACT queue never blocks waiting for this tile's main matmuls
                if prev_psm is not None:
                    _exp_reduce(nc, mybir, expp, prev_psm, sums_sb, t - 1, bf16, AF)
                prev_psm = psm

            _exp_reduce(nc, mybir, expp, prev_psm, sums_sb, NTILES - 1, bf16, AF)

            # epilogue: ll^T = Ln(sums); transpose; contiguous DMA out
            llT = finp.tile([128, NTILES * 4], f32)
            nc.scalar.activation(llT[:], sums_sb[:], AF.Ln)
            llps = ps_sums.tile([128, TILE_ROWS], f32, tag="sm")
            nc.tensor.transpose(llps[:, 0:128], llT[:], EYE_sb[:])
            ll_sb = finp.tile([128, 128], f32)
            nc.scalar.copy(ll_sb[:], llps[:, 0:128])
            nc.sync.dma_start(OUT_d.rearrange("(c p) -> c p", c=128), ll_sb[:])

    nc.compile()
    return nc


def _host_prep(center, cov_inv_sqrt, weight, threshold):
    L = np.asarray(cov_inv_sqrt, dtype=np.float64)
    w = np.abs(np.asarray(weight, dtype=np.float64))
    pr = w / w.sum()
    A = np.einsum("mij,mkj->mik", L, L)
    sign, logdet = np.linalg.slogdet(A)
    logcoef = np.log(pr) + 0.5 * logdet
    c64 = np.asarray(center, dtype=np.float64)
    Ac = np.einsum("mkl,ml->mk", A, c64)
    term3 = np.einsum("mk,mk->m", c64, Ac)
    bias = logcoef - 0.5 * term3 - float(np.asarray(threshold).reshape(-1)[0])

    U = np.zeros((128, NCHUNK, M), np.float64)
    p = np.arange(128)
    d = p % 32
    rowsum = A.sum(axis=2)                              # [M, 32]
    a16 = A[:, np.arange(32), (np.arange(32) + 16) % 32]
    for c in range(4):
        k = 4 * c + p // 32
        b = (d + k) % 32
        coef = -0.5 * A[:, d, b]                        # [M, 128]
        diagc = -0.5 * (2.0 * A[:, d, d] - rowsum[:, d] + a16[:, d]) / 4.0
        U[:, c, :] = np.where((k == 0)[None, :], diagc, coef).T
    d16 = np.arange(16)
    U[0:16, 4, :] = (-A[:, d16, d16 + 16]).T
    U[32:64, 4, :] = Ac.T
    U[64, 4, :] = bias

    SEL = np.zeros((33, 4, 128), np.float32)
    for c in range(4):
        k = 4 * c + p // 32
        b = (d + k) % 32
        for pp in range(128):
            SEL[d[pp], c, pp] += 1.0
            SEL[b[pp], c, pp] += 1.0

    Z16 = np.zeros((16, TILE_ROWS), BF16)
    ONE1 = np.ones((1, TILE_ROWS), BF16)
    EYE = np.eye(128, dtype=np.float32)
    return (
        U.astype(BF16),
        SEL.astype(BF16),
        Z16,
        ONE1,
        EYE,
    )


def build_in_maps(X, center, cov_inv_sqrt, weight, threshold):
    X = np.ascontiguousarray(np.asarray(X, dtype=np.float32))
    U, SEL, Z16, ONE1, EYE = _host_prep(center, cov_inv_sqrt, weight, threshold)
    in_maps = []
    for k in range(NCORES):
        XT = np.ascontiguousarray(
            X[k * NC_ROWS : (k + 1) * NC_ROWS].T.astype(BF16)
        )
        in_maps.append(
            {"XT": XT, "U": U, "SEL": SEL, "Z16": Z16, "ONE1": ONE1, "EYE": EYE}
        )
    return in_maps


def kernel(X, center, cov_inv_sqrt, weight, threshold):
    global _PROGRAM
    from concourse.bass_utils import run_bass_kernel_spmd

    in_maps = build_in_maps(X, center, cov_inv_sqrt, weight, threshold)

    if _PROGRAM is None:
        _PROGRAM = _build_program()
    nc = _PROGRAM

    res = run_bass_kernel_spmd(nc, in_maps, list(range(NCORES)))
    out = np.concatenate([res.results[k]["out"] for k in range(NCORES)])
    return out.astype(np.float32)


# revision 15
# speedup vs baseline: 1.0219x; 1.0219x over previous
"""Trainium2 Bass kernel for nn_DetectorKe_652835029279 (Gaussian-mixture
log-likelihood detector: weighted logsumexp over 256 Mahalanobis distances).

v2 "squares basis": ll_i = logsumexp_j(-0.5 x^T A_j x + x.(A_j c_j) + bias_j)
with the quadratic form expanded in the basis
  (x_a + x_b)^2  for pairs at circular distance k=1..15  (4 chunks of 128)
  (2 x_d)^2      for the diagonal (k=0 slots)
  x_d * x_{d+16} for the 16 distance-16 pairs            (chunk 4, rows 0:16)
  x_d, 1         linear + bias rows                      (chunk 4, rows 32:65)
so the selection matmuls produce SUMS x_a+x_b directly (2-hot SEL), the
elementwise step is a unary SQUARE (splittable between ACT and DVE), and the
distance-16 pairs come from one cheap DVE bf16 multiply of two SBUF tiles.
The whole pipeline is bf16 (FWL hides LDWEIGHTS behind the matmul stream);
X arrives pre-transposed [32, N] in bf16 from the host, which removes all
PE transposes. d' = G^T @ U with 5 chunks (4x K=128 + 1x K=65): 20 main
matmuls of N=256 + 4 selection matmuls of N=512 per 512-row tile.

Per tile: 3 DMAs (X^T slab + two 16-row slices) -> DVE k16-product ->
4 SEL matmuls (PSUM) -> squares (3 on ACT, 1 on DVE) -> 20 accumulating
matmuls into one [128,1024] PSUM tile -> ACT exp (bf16) -> DVE free-dim
reduce into per-subtile sums -> (end) Ln + PE transpose + DMA out.
PSUM budget: 4 banks sel-sums + 2x2 banks main = 8.
"""
import sys

if "/opt/trn_rl_repo" not in sys.path:
    sys.path.insert(0, "/opt/trn_rl_repo")

import numpy as np
import ml_dtypes

BF16 = ml_dtypes.bfloat16

N, D, M = 131072, 32, 256
NCORES = 8
NC_ROWS = N // NCORES          # 16384
TILE_ROWS = 512
NTILES = NC_ROWS // TILE_ROWS  # 32
NCHUNK = 5

_PROGRAM = None


def _build_program():
    import concourse.bacc as bacc
    import concourse.mybir as mybir
    import concourse.tile as tile

    f32 = mybir.dt.float32
    bf16 = mybir.dt.bfloat16
    AF = mybir.ActivationFunctionType

    def _exp_reduce(nc, mybir, expp, psm, sums_sb, t, bf16, AF):
        ex = expp.tile([128, 4 * M], bf16, tag="exp")
        nc.scalar.activation(ex[:], psm[:], AF.Exp)
        with nc.allow_low_precision("bf16 exp sums; 0.4% rel on ll is in budget"):
            nc.vector.tensor_reduce(
                sums_sb[:, 4 * t : 4 * t + 4],
                ex[:].rearrange("p (s m) -> p s m", s=4),
                axis=mybir.AxisListType.X,
                op=mybir.AluOpType.add,
            )

    nc = bacc.Bacc(None, target_bir_lowering=False)
    XT_d = nc.dram_tensor("XT", [D, NC_ROWS], bf16, kind="ExternalInput")
    U_d = nc.dram_tensor("U", [128, NCHUNK, M], bf16, kind="ExternalInput")
    SEL_d = nc.dram_tensor("SEL", [33, 4, 128], bf16, kind="ExternalInput")
    Z16_d = nc.dram_tensor("Z16", [16, TILE_ROWS], bf16, kind="ExternalInput")
    ONE1_d = nc.dram_tensor("ONE1", [1, TILE_ROWS], bf16, kind="ExternalInput")
    EYE_d = nc.dram_tensor("EYE", [128, 128], f32, kind="ExternalInput")
    OUT_d = nc.dram_tensor("out", [NC_ROWS], f32, kind="ExternalOutput")

    with tile.TileContext(nc) as tc:
        with (
            tc.tile_pool(name="const", bufs=1) as constp,
            tc.tile_pool(name="c4", bufs=2) as c4pool,
            tc.tile_pool(name="xlo", bufs=2) as xlop,
            tc.tile_pool(name="xhi", bufs=2) as xhip,
            tc.tile_pool(name="ch", bufs=8) as chp,
            tc.tile_pool(name="expp", bufs=2) as expp,
            tc.tile_pool(name="sums", bufs=1) as sumsp,
            tc.tile_pool(name="fin", bufs=1) as finp,
            tc.tile_pool(name="ps_sums", bufs=4, space="PSUM") as ps_sums,
            tc.tile_pool(name="ps_main", bufs=2, space="PSUM") as ps_main,
        ):
            # SEL first: the sel matmuls are the first PE work and should not
            # queue behind the larger U/EYE transfers
            SEL_sb = constp.tile([33, 4, 128], bf16)
            nc.sync.dma_start(SEL_sb[:], SEL_d[:])

            sums_sb = sumsp.tile([128, NTILES * 4], bf16)

            # persistent chunk-4 tiles: rows 0:16 = x_d*x_{d+16} (per tile),
            # rows 16:32 = zeros (once), rows 32:64 = X^T (DMA per tile),
            # row 64 = ones (once). Main matmul reads K=0:65.
            c4_tiles = []
            for i in range(2):
                c4t = c4pool.tile(
                    [128, TILE_ROWS], bf16, tag=f"c4{i}", bufs=1, name=f"c4_{i}"
                )
                nc.sync.dma_start(c4t[16:32, :], Z16_d[:])
                nc.sync.dma_start(c4t[64:65, :], ONE1_d[:])
                c4_tiles.append(c4t)

            # sel-matmul moving operand at base partition 0: [X^T; ones]
            xsel_tiles = []
            for i in range(2):
                xs = c4pool.tile(
                    [33, TILE_ROWS], bf16, tag=f"xs{i}", bufs=1, name=f"xs_{i}"
                )
                nc.sync.dma_start(xs[32:33, :], ONE1_d[:])
                xsel_tiles.append(xs)

            # larger constants after the first-tile critical path
            U_sb = constp.tile([128, NCHUNK, M], bf16)
            nc.sync.dma_start(U_sb[:], U_d[:])
            EYE_sb = constp.tile([128, 128], f32)
            nc.sync.dma_start(EYE_sb[:], EYE_d[:])

            prev_psm = None
            for t in range(NTILES):
                c4t = c4_tiles[t % 2]
                xs = xsel_tiles[t % 2]
                lo = t * TILE_ROWS
                nc.sync.dma_start(c4t[32:64, :], XT_d[:, lo : lo + TILE_ROWS])
                nc.sync.dma_start(xs[0:32, :], XT_d[:, lo : lo + TILE_ROWS])
                xlo = xlop.tile([16, TILE_ROWS], bf16, tag="xlo")
                nc.sync.dma_start(xlo[:], XT_d[0:16, lo : lo + TILE_ROWS])
                xhi = xhip.tile([16, TILE_ROWS], bf16, tag="xhi")
                nc.sync.dma_start(xhi[:], XT_d[16:32, lo : lo + TILE_ROWS])

                # distance-16 pair products (bf16 2x DVE)
                nc.vector.tensor_mul(c4t[0:16, :], xlo[:], xhi[:])

                # selection matmuls -> (x_a + x_b) sums in PSUM, then square
                ch_tiles = []
                for c in range(4):
                    sm = ps_sums.tile([128, TILE_ROWS], f32, tag="sm")
                    nc.tensor.matmul(
                        sm[:], SEL_sb[:, c, :], xs[:], start=True, stop=True
                    )
                    ch = chp.tile([128, TILE_ROWS], bf16, tag="ch")
                    if c == 3:
                        # DVE has one PSUM read port: copy to bf16 SBUF first,
                        # then square in 2x bf16 mode.
                        tmp = chp.tile([128, TILE_ROWS], bf16, tag="sq3tmp", bufs=2)
                        nc.vector.tensor_copy(tmp[:], sm[:])
                        nc.vector.tensor_mul(ch[:], tmp[:], tmp[:])
                    else:
                        nc.scalar.activation(ch[:], sm[:], AF.Square)
                    ch_tiles.append(ch)

                # main accumulating matmuls: 4 row-subtiles x 5 chunks
                psm = ps_main.tile([128, 4 * M], f32, tag="main")
                for sub in range(4):
                    for c in range(4):
                        nc.tensor.matmul(
                            psm[:, sub * M : (sub + 1) * M],
                            ch_tiles[c][:, sub * 128 : (sub + 1) * 128],
                            U_sb[:, c, :],
                            start=(c == 0),
                            stop=False,
                        )
                    nc.tensor.matmul(
                        psm[:, sub * M : (sub + 1) * M],
                        c4t[0:65, sub * 128 : (sub + 1) * 128],
                        U_sb[0:65, 4, :],
                        start=False,
                        stop=True,
                    )

                # software-pipelined epilogue: exp/reduce for tile t-1, so the
                # ACT queue never blocks waiting for this tile's main matmuls
                if prev_psm is not None:
                    _exp_reduce(nc, mybir, expp, prev_psm, sums_sb, t - 1, bf16, AF)
                prev_psm = psm

            _exp_reduce(nc, mybir, expp, prev_psm, sums_sb, NTILES - 1, bf16, AF)

            # epilogue: ll^T = Ln(sums); transpose; contiguous DMA out
            llT = finp.tile([128, NTILES * 4], f32)
            nc.scalar.activation(llT[:], sums_sb[:], AF.Ln)
            llps = ps_sums.tile([128, TILE_ROWS], f32, tag="sm")
            nc.tensor.transpose(llps[:, 0:128], llT[:], EYE_sb[:])
            ll_sb = finp.tile([128, 128], f32)
            nc.scalar.copy(ll_sb[:], llps[:, 0:128])
            nc.sync.dma_start(OUT_d.rearrange("(c p) -> c p", c=128), ll_sb[:])

    nc.compile()
    return nc


def _host_prep(center, cov_inv_sqrt, weight, threshold):
    L = np.asarray(cov_inv_sqrt, dtype=np.float64)
    w = np.abs(np.asarray(weight, dtype=np.float64))
    pr = w / w.sum()
    A = np.einsum("mij,mkj->mik", L, L)
    sign, logdet = np.linalg.slogdet(A)
    logcoef = np.log(pr) + 0.5 * logdet
    c64 = np.asarray(center, dtype=np.float64)
    Ac = np.einsum("mkl,ml->mk", A, c64)
    term3 = np.einsum("mk,mk->m", c64, Ac)
    bias = logcoef - 0.5 * term3 - float(np.asarray(threshold).reshape(-1)[0])

    U = np.zeros((128, NCHUNK, M), np.float64)
    p = np.arange(128)
    d = p % 32
    rowsum = A.sum(axis=2)                              # [M, 32]
    a16 = A[:, np.arange(32), (np.arange(32) + 16) % 32]
    for c in range(4):
        k = 4 * c + p // 32
        b = (d + k) % 32
        coef = -0.5 * A[:, d, b]                        # [M, 128]
        diagc = -0.5 * (2.0 * A[:, d, d] - rowsum[:, d] + a16[:, d]) / 4.0
        U[:, c, :] = np.where((k == 0)[None, :], diagc, coef).T
    d16 = np.arange(16)
    U[0:16, 4, :] = (-A[:, d16, d16 + 16]).T
    U[32:64, 4, :] = Ac.T
    U[64, 4, :] = bias

    SEL = np.zeros((33, 4, 128), np.float32)
    for c in range(4):
        k = 4 * c + p // 32
        b = (d + k) % 32
        for pp in range(128):
            SEL[d[pp], c, pp] += 1.0
            SEL[b[pp], c, pp] += 1.0

    Z16 = np.zeros((16, TILE_ROWS), BF16)
    ONE1 = np.ones((1, TILE_ROWS), BF16)
    EYE = np.eye(128, dtype=np.float32)
    return (
        U.astype(BF16),
        SEL.astype(BF16),
        Z16,
        ONE1,
        EYE,
    )


def build_in_maps(X, center, cov_inv_sqrt, weight, threshold):
    X = np.ascontiguousarray(np.asarray(X, dtype=np.float32))
    U, SEL, Z16, ONE1, EYE = _host_prep(center, cov_inv_sqrt, weight, threshold)
    in_maps = []
    for k in range(NCORES):
        XT = np.ascontiguousarray(
            X[k * NC_ROWS : (k + 1) * NC_ROWS].T.astype(BF16)
        )
        in_maps.append(
            {"XT": XT, "U": U, "SEL": SEL, "Z16": Z16, "ONE1": ONE1, "EYE": EYE}
        )
    return in_maps


def kernel(X, center, cov_inv_sqrt, weight, threshold):
    global _PROGRAM
    from concourse.bass_utils import run_bass_kernel_spmd

    in_maps = build_in_maps(X, center, cov_inv_sqrt, weight, threshold)

    if _PROGRAM is None:
        _PROGRAM = _build_program()
    nc = _PROGRAM

    res = run_bass_kernel_spmd(nc, in_maps, list(range(NCORES)))
    out = np.concatenate([res.results[k]["out"] for k in range(NCORES)])
    return out.astype(np.float32)


# revision 16
# speedup vs baseline: 1.0433x; 1.0209x over previous
"""Trainium2 Bass kernel for nn_DetectorKe_652835029279 (Gaussian-mixture
log-likelihood detector: weighted logsumexp over 256 Mahalanobis distances).

v2 "squares basis": ll_i = logsumexp_j(-0.5 x^T A_j x + x.(A_j c_j) + bias_j)
with the quadratic form expanded in the basis
  (x_a + x_b)^2  for pairs at circular distance k=1..15  (4 chunks of 128)
  (2 x_d)^2      for the diagonal (k=0 slots)
  x_d * x_{d+16} for the 16 distance-16 pairs            (chunk 4, rows 0:16)
  x_d, 1         linear + bias rows                      (chunk 4, rows 32:65)
so the selection matmuls produce SUMS x_a+x_b directly (2-hot SEL), the
elementwise step is a unary SQUARE (splittable between ACT and DVE), and the
distance-16 pairs come from one cheap DVE bf16 multiply of two SBUF tiles.
The whole pipeline is bf16 (FWL hides LDWEIGHTS behind the matmul stream);
X arrives pre-transposed [32, N] in bf16 from the host, which removes all
PE transposes. d' = G^T @ U with 5 chunks (4x K=128 + 1x K=65): 20 main
matmuls of N=256 + 4 selection matmuls of N=512 per 512-row tile.

Per tile: 3 DMAs (X^T slab + two 16-row slices) -> DVE k16-product ->
4 SEL matmuls (PSUM) -> squares (3 on ACT, 1 on DVE) -> 20 accumulating
matmuls into one [128,1024] PSUM tile -> ACT exp (bf16) -> DVE free-dim
reduce into per-subtile sums -> (end) Ln + PE transpose + DMA out.
PSUM budget: 4 banks sel-sums + 2x2 banks main = 8.
"""
import sys

if "/opt/trn_rl_repo" not in sys.path:
    sys.path.insert(0, "/opt/trn_rl_repo")

import numpy as np
import ml_dtypes

BF16 = ml_dtypes.bfloat16

N, D, M = 131072, 32, 256
NCORES = 8
NC_ROWS = N // NCORES          # 16384
TILE_ROWS = 512
NTILES = NC_ROWS // TILE_ROWS  # 32
NCHUNK = 5

_PROGRAM = None


def _build_program():
    import concourse.bacc as bacc
    import concourse.mybir as mybir
    import concourse.tile as tile

    f32 = mybir.dt.float32
    bf16 = mybir.dt.bfloat16
    AF = mybir.ActivationFunctionType

    def _exp_reduce(nc, mybir, expp, psm, sums_sb, t, bf16, AF):
        ex = expp.tile([128, 4 * M], bf16, tag="exp")
        nc.scalar.activation(ex[:], psm[:], AF.Exp)
        with nc.allow_low_precision("bf16 exp sums; 0.4% rel on ll is in budget"):
            nc.vector.tensor_reduce(
                sums_sb[:, 4 * t : 4 * t + 4],
                ex[:].rearrange("p (s m) -> p s m", s=4),
                axis=mybir.AxisListType.X,
                op=mybir.AluOpType.add,
            )

    nc = bacc.Bacc(None, target_bir_lowering=False)
    XT_d = nc.dram_tensor("XT", [D, NC_ROWS], bf16, kind="ExternalInput")
    U_d = nc.dram_tensor("U", [128, NCHUNK, M], bf16, kind="ExternalInput")
    SEL_d = nc.dram_tensor("SEL", [33, 4, 128], bf16, kind="ExternalInput")
    Z16_d = nc.dram_tensor("Z16", [16, TILE_ROWS], bf16, kind="ExternalInput")
    ONE1_d = nc.dram_tensor("ONE1", [1, TILE_ROWS], bf16, kind="ExternalInput")
    EYE_d = nc.dram_tensor("EYE", [128, 128], f32, kind="ExternalInput")
    OUT_d = nc.dram_tensor("out", [NC_ROWS], f32, kind="ExternalOutput")

    with tile.TileContext(nc) as tc:
        with (
            tc.tile_pool(name="const", bufs=1) as constp,
            tc.tile_pool(name="c4", bufs=2) as c4pool,
            tc.tile_pool(name="xlo", bufs=2) as xlop,
            tc.tile_pool(name="xhi", bufs=2) as xhip,
            tc.tile_pool(name="ch", bufs=8) as chp,
            tc.tile_pool(name="expp", bufs=2) as expp,
            tc.tile_pool(name="sums", bufs=1) as sumsp,
            tc.tile_pool(name="fin", bufs=1) as finp,
            tc.tile_pool(name="ps_sums", bufs=4, space="PSUM") as ps_sums,
            tc.tile_pool(name="ps_main", bufs=2, space="PSUM") as ps_main,
        ):
            # SEL first: the sel matmuls are the first PE work and should not
            # queue behind the larger U/EYE transfers
            SEL_sb = constp.tile([33, 4, 128], bf16)
            nc.sync.dma_start(SEL_sb[:], SEL_d[:])

            sums_sb = sumsp.tile([128, NTILES * 4], bf16)

            # persistent chunk-4 tiles: rows 0:16 = x_d*x_{d+16} (per tile),
            # rows 16:32 = zeros (once), rows 32:64 = X^T (DMA per tile),
            # row 64 = ones (once). Main matmul reads K=0:65.
            c4_tiles = []
            for i in range(2):
                c4t = c4pool.tile(
                    [128, TILE_ROWS], bf16, tag=f"c4{i}", bufs=1, name=f"c4_{i}"
                )
                nc.sync.dma_start(c4t[16:32, :], Z16_d[:])
                nc.sync.dma_start(c4t[64:65, :], ONE1_d[:])
                c4_tiles.append(c4t)

            # sel-matmul moving operand at base partition 0: [X^T; ones]
            xsel_tiles = []
            for i in range(2):
                xs = c4pool.tile(
                    [33, TILE_ROWS], bf16, tag=f"xs{i}", bufs=1, name=f"xs_{i}"
                )
                nc.sync.dma_start(xs[32:33, :], ONE1_d[:])
                xsel_tiles.append(xs)

            # larger constants after the first-tile critical path
            U_sb = constp.tile([128, NCHUNK, M], bf16)
            nc.sync.dma_start(U_sb[:], U_d[:])
            EYE_sb = constp.tile([128, 128], f32)
            nc.sync.dma_start(EYE_sb[:], EYE_d[:])

            prev_psm = None
            for t in range(NTILES):
                c4t = c4_tiles[t % 2]
                xs = xsel_tiles[t % 2]
                lo = t * TILE_ROWS
                nc.sync.dma_start(c4t[32:64, :], XT_d[:, lo : lo + TILE_ROWS])
                nc.sync.dma_start(xs[0:32, :], XT_d[:, lo : lo + TILE_ROWS])
                xlo = xlop.tile([16, TILE_ROWS], bf16, tag="xlo")
                nc.sync.dma_start(xlo[:], XT_d[0:16, lo : lo + TILE_ROWS])
                xhi = xhip.tile([16, TILE_ROWS], bf16, tag="xhi")
                nc.sync.dma_start(xhi[:], XT_d[16:32, lo : lo + TILE_ROWS])

                # distance-16 pair products (bf16 2x DVE)
                nc.vector.tensor_mul(c4t[0:16, :], xlo[:], xhi[:])

                # selection matmuls -> (x_a + x_b) sums in PSUM, then square
                ch_tiles = []
                for c in range(4):
                    sm = ps_sums.tile([128, TILE_ROWS], f32, tag="sm")
                    nc.tensor.matmul(
                        sm[:], SEL_sb[:, c, :], xs[:], start=True, stop=True
                    )
                    ch = chp.tile([128, TILE_ROWS], bf16, tag="ch")
                    if c == 3:
                        # DVE has one PSUM read port: copy to bf16 SBUF first,
                        # then square in 2x bf16 mode.
                        tmp = chp.tile([128, TILE_ROWS], bf16, tag="sq3tmp", bufs=2)
                        nc.vector.tensor_copy(tmp[:], sm[:])
                        nc.vector.tensor_mul(ch[:], tmp[:], tmp[:])
                    else:
                        nc.scalar.activation(ch[:], sm[:], AF.Square)
                    ch_tiles.append(ch)

                # main accumulating matmuls: 4 row-subtiles x 5 chunks
                psm = ps_main.tile([128, 4 * M], f32, tag="main")
                for sub in range(4):
                    for c in range(4):
                        nc.tensor.matmul(
                            psm[:, sub * M : (sub + 1) * M],
                            ch_tiles[c][:, sub * 128 : (sub + 1) * 128],
                            U_sb[:, c, :],
                            start=(c == 0),
                            stop=False,
                        )
                    nc.tensor.matmul(
                        psm[:, sub * M : (sub + 1) * M],
                        c4t[0:65, sub * 128 : (sub + 1) * 128],
                        U_sb[0:65, 4, :],
                        start=False,
                        stop=True,
                    )

                _exp_reduce(nc, mybir, expp, psm, sums_sb, t, bf16, AF)

            # epilogue: ll^T = Ln(sums); transpose; contiguous DMA out
            llT = finp.tile([128, NTILES * 4], f32)
            nc.scalar.activation(llT[:], sums_sb[:], AF.Ln)
            llps = ps_sums.tile([128, TILE_ROWS], f32, tag="sm")
            nc.tensor.transpose(llps[:, 0:128], llT[:], EYE_sb[:])
            ll_sb = finp.tile([128, 128], f32)
            nc.scalar.copy(ll_sb[:], llps[:, 0:128])
            nc.sync.dma_start(OUT_d.rearrange("(c p) -> c p", c=128), ll_sb[:])

    nc.compile()
    return nc


def _host_prep(center, cov_inv_sqrt, weight, threshold):
    L = np.asarray(cov_inv_sqrt, dtype=np.float64)
    w = np.abs(np.asarray(weight, dtype=np.float64))
    pr = w / w.sum()
    A = np.einsum("mij,mkj->mik", L, L)
    sign, logdet = np.linalg.slogdet(A)
    logcoef = np.log(pr) + 0.5 * logdet
    c64 = np.asarray(center, dtype=np.float64)
    Ac = np.einsum("mkl,ml->mk", A, c64)
    term3 = np.einsum("mk,mk->m", c64, Ac)
    bias = logcoef - 0.5 * term3 - float(np.asarray(threshold).reshape(-1)[0])

    U = np.zeros((128, NCHUNK, M), np.float64)
    p = np.arange(128)
    d = p % 32
    rowsum = A.sum(axis=2)                              # [M, 32]
    a16 = A[:, np.arange(32), (np.arange(32) + 16) % 32]
    for c in range(4):
        k = 4 * c + p // 32
        b = (d + k) % 32
        coef = -0.5 * A[:, d, b]                        # [M, 128]
        diagc = -0.5 * (2.0 * A[:, d, d] - rowsum[:, d] + a16[:, d]) / 4.0
        U[:, c, :] = np.where((k == 0)[None, :], diagc, coef).T
    d16 = np.arange(16)
    U[0:16, 4, :] = (-A[:, d16, d16 + 16]).T
    U[32:64, 4, :] = Ac.T
    U[64, 4, :] = bias

    SEL = np.zeros((33, 4, 128), np.float32)
    for c in range(4):
        k = 4 * c + p // 32
        b = (d + k) % 32
        for pp in range(128):
            SEL[d[pp], c, pp] += 1.0
            SEL[b[pp], c, pp] += 1.0

    Z16 = np.zeros((16, TILE_ROWS), BF16)
    ONE1 = np.ones((1, TILE_ROWS), BF16)
    EYE = np.eye(128, dtype=np.float32)
    return (
        U.astype(BF16),
        SEL.astype(BF16),
        Z16,
        ONE1,
        EYE,
    )


def build_in_maps(X, center, cov_inv_sqrt, weight, threshold):
    X = np.ascontiguousarray(np.asarray(X, dtype=np.float32))
    U, SEL, Z16, ONE1, EYE = _host_prep(center, cov_inv_sqrt, weight, threshold)
    in_maps = []
    for k in range(NCORES):
        XT = np.ascontiguousarray(
            X[k * NC_ROWS : (k + 1) * NC_ROWS].T.astype(BF16)
        )
        in_maps.append(
            {"XT": XT, "U": U, "SEL": SEL, "Z16": Z16, "ONE1": ONE1, "EYE": EYE}
        )
    return in_maps


def kernel(X, center, cov_inv_sqrt, weight, threshold):
    global _PROGRAM
    from concourse.bass_utils import run_bass_kernel_spmd

    in_maps = build_in_maps(X, center, cov_inv_sqrt, weight, threshold)

    if _PROGRAM is None:
        _PROGRAM = _build_program()
    nc = _PROGRAM

    res = run_bass_kernel_spmd(nc, in_maps, list(range(NCORES)))
    out = np.concatenate([res.results[k]["out"] for k in range(NCORES)])
    return out.astype(np.float32)
